# revision 18
# baseline (speedup 1.0000x reference)
"""GRU cell kernel for Trainium2, data-parallel over 8 NeuronCores.

Math (per reference):
    z = sigmoid(x @ wz.T + h @ uz.T + bz)
    r = sigmoid(x @ wr.T + h @ ur.T + br)
    g = tanh(x @ wh.T + (r*h) @ uh.T + bh)
    out = (1-z)*h + z*g = h + z*(g - h)

Everything on-device is computed in TRANSPOSED layout ([feature, row]) so
both matmul operands arrive with the contraction dim on partitions.

Mixed precision: part of the contraction runs as fp8(e4m3) DoubleRow
matmuls (2 MACs/cell/cycle, K=256 per pass), the rest as bf16. Which
k-quarters of each weight matrix are fp8 was chosen by host simulation
to keep max rel err ~0.014 (budget 2e-2):
    wr, ur, uh: all 4 quarters fp8 (r-gate error is attenuated by
        sigmoid slope and the uh moving operand r*h is small in
        magnitude)
    wz, uz, wh: quarter 0 fp8, quarters 1..3 bf16 (z-gate errors are
        amplified by (g - h), tanh has slope 1)
All weights (fp8 and bf16) are pre-scaled by 32 on host (exact in both
formats) so fp8 and bf16 products can share one PSUM accumulation; the
activation undoes it with scale=1/32.

Sharding: rows 16384 -> 8 cores x 2048 rows, weights replicated.
"""

import numpy as np
import ml_dtypes
from contextlib import ExitStack

import concourse.bass as bass
import concourse.bacc as bacc
import concourse.mybir as mybir
import concourse.tile as tile
from concourse.bass_utils import run_bass_kernel_spmd

H = 1024
N_ROWS = 16384
NCORES = 8
P = 128
KB = H // P            # 8 contraction blocks of 128
MB = H // P            # 8 output-feature blocks
NQ = 4                 # k-quarters (256 each)
NS = 512               # rows per matmul moving slice (one PSUM bank)
WSCALE = 32.0          # weight pre-scale (exact power of 2)

# fp8 k-quarters per weight matrix (first nq of 4 quarters are fp8)
NQ8 = {"wz": 1, "uz": 1, "wr": 4, "ur": 4, "wh": 2, "uh": 4}

BF = mybir.dt.bfloat16
F8 = mybir.dt.float8e4
F32 = mybir.dt.float32
AF = mybir.ActivationFunctionType
DR = mybir.MatmulPerfMode.DoubleRow
bf16 = ml_dtypes.bfloat16
f8 = ml_dtypes.float8_e4m3

# Set by test harness to capture a trace; harness-facing default off.
TRACE = False
LAST_RESULT = None


def build_nc(R=N_ROWS // NCORES):
    """Build the per-core Bass program. R rows per core, single chunk."""
    SL = R // NS           # moving slices (4 for R=2048)

    nc = bacc.Bacc(trn_type="TRN2", target_bir_lowering=False,
                   debug=False, enable_asserts=False)

    # All block tensors use "partition-major block layout": [128, nblk*cols]
    # with element (p, k*cols + c) = T[k*128 + p, c]. One DMA descriptor can
    # then cover any k-block range (descriptor processing on the queue
    # engines costs ~0.65us each — fine-grained DMA was the startup limiter).
    x8d = nc.dram_tensor("x8", [P, KB * R], F8, kind="ExternalInput").ap()
    xbd = nc.dram_tensor("xb", [P, 6 * R], BF, kind="ExternalInput").ap()
    h8d = nc.dram_tensor("h8", [P, KB * R], F8, kind="ExternalInput").ap()
    hbd = nc.dram_tensor("hb", [P, 6 * R], BF, kind="ExternalInput").ap()
    hfd = nc.dram_tensor("hf", [H, R], F32, kind="ExternalInput").ap()
    w8d = {}
    wbd = {}
    for nm, nq in NQ8.items():
        w8d[nm] = nc.dram_tensor(nm + "8", [P, nq * 2 * H], F8,
                                 kind="ExternalInput").ap()
        if nq < NQ:
            wbd[nm] = nc.dram_tensor(nm + "b", [P, (NQ - nq) * 2 * H], BF,
                                     kind="ExternalInput").ap()
    bias = nc.dram_tensor("bias", [P, 3 * MB], F32, kind="ExternalInput").ap()
    outT = nc.dram_tensor("outT", [H, R], BF, kind="ExternalOutput").ap()

    with tile.TileContext(nc) as tc, ExitStack() as ctx:
        wpool = ctx.enter_context(tc.tile_pool(name="w", bufs=3))
        dpool = ctx.enter_context(tc.tile_pool(name="d", bufs=1))
        hfpool = ctx.enter_context(tc.tile_pool(name="hf", bufs=2))
        rpool = ctx.enter_context(tc.tile_pool(name="r", bufs=3))
        gpool = ctx.enter_context(tc.tile_pool(name="g", bufs=3))
        dtpool = ctx.enter_context(tc.tile_pool(name="dt", bufs=SL))
        opool = ctx.enter_context(tc.tile_pool(name="o", bufs=4))
        cpool = ctx.enter_context(tc.tile_pool(name="c", bufs=1))
        pspool = ctx.enter_context(tc.tile_pool(name="ps", bufs=8, space="PSUM"))

        # Two HW DMA queues: sync (bulk prefetch + output) and scalar
        # (x8/h8 at startup, then the latency-sensitive hf stream). Keeping
        # the hf stream off the bulk queue is essential — behind the 12MB
        # hz prefetch it starves the r-pass rh-multiply for ~30us.
        bt = cpool.tile([P, 3 * MB], F32, tag="bias")
        nc.scalar.dma_start(bt[:], bias[:])
        # bias column layout: [z:0..7 | r:8..15 | h:16..23]
        GZ, GR, GH = 0, 1, 2
        ISC = 1.0 / WSCALE

        # ---- SBUF data tiles + DMA in consumption order ----
        # r-pass m=0 consumes wr/x first, then ur/h; interleave so the PE
        # can start as soon as the first (weight, data) pair lands.
        xt8 = dpool.tile([P, KB, R], F8, tag="x8")
        ht8 = dpool.tile([P, KB, R], F8, tag="h8")
        xtb = dpool.tile([P, 6, R], BF, tag="xb")
        htb = dpool.tile([P, 6, R], BF, tag="hb")
        rht = dpool.tile([P, KB, R], F8, tag="rh")

        w8t = {}
        wbt = {}
        # Critical path (r-pass m=0): kq-granular (0.25-0.5MB) descriptors,
        # weights on the sync queue, x8/h8 on the scalar queue, in the exact
        # consumption order of the first psum group.
        # Critical 6MB split evenly across the two HW queues (each delivers
        # ~270GB/s): sync gets wr+h8, scalar gets x8+ur, both in the exact
        # consumption order of the first psum group.
        w8t["wr"] = wpool.tile([P, KB, H], F8, tag="w8", name="wr8", bufs=2)
        w8t["ur"] = wpool.tile([P, KB, H], F8, tag="w8", name="ur8", bufs=2)
        for kq in range(NQ):
            j = slice(2 * kq, 2 * kq + 2)
            nc.sync.dma_start(w8t["wr"][:, j, :], w8d["wr"][:, 2 * kq * H:(2 * kq + 2) * H])
            nc.scalar.dma_start(xt8[:, j, :], x8d[:, 2 * kq * R:(2 * kq + 2) * R])
            nc.sync.dma_start(ht8[:, j, :], h8d[:, 2 * kq * R:(2 * kq + 2) * R])
            nc.scalar.dma_start(w8t["ur"][:, j, :], w8d["ur"][:, 2 * kq * H:(2 * kq + 2) * H])

        # Warm up the ACT table set (sigmoid_and_others covers tanh too) on an
        # instruction with minimal sync waits — walrus can't attach the
        # PSEUDO_LOAD_ACT_FUNC_SET to an activation that already carries two
        # sem waits ("Too many sync wait commands"). Emitted after the
        # critical DMAs so the 2x1.3us table loads don't delay them.
        warm = cpool.tile([P, 8], F32, tag="warm")
        nc.gpsimd.memset(warm[:], 0.0)
        nc.scalar.activation(warm[:], warm[:], AF.Sigmoid)

        # hz-pass weights + bf16 moving data: one descriptor per tensor,
        # streamed during the r-pass on the sync queue.
        for nm in ("wh", "wz", "uz"):
            nq = NQ8[nm]
            w8t[nm] = wpool.tile([P, 2 * nq, H], F8, tag="w8q", name=nm + "8")
            nc.sync.dma_start(w8t[nm][:, :, :], w8d[nm][:, :])
            wbt[nm] = wpool.tile([P, 2 * (NQ - nq), H], BF, tag="wbq", name=nm + "b")
            nc.sync.dma_start(wbt[nm][:, :, :], wbd[nm][:, :])
        nc.sync.dma_start(xtb[:, :, :], xbd[:, :])
        nc.sync.dma_start(htb[:, :, :], hbd[:, :])
        # uh8 reuses wr's buffer (tag w8, bufs=2): its DMA waits for the
        # r-pass to drain, so it must sit at the TAIL of the sync queue to
        # avoid head-of-line blocking the prefetch above.
        w8t["uh"] = wpool.tile([P, KB, H], F8, tag="w8", name="uh8", bufs=2)
        nc.sync.dma_start(w8t["uh"][:, :, :], w8d["uh"][:, :])


        def mm_fp8(psums, wt, mov, m, nq, start, stop):
            """DoubleRow-accumulate wt.T @ mov for feature block m over
            fp8 k-quarters 0..nq-1."""
            msl = slice(m * P, (m + 1) * P)
            for kq in range(nq):
                for s in range(SL):
                    nc.tensor.matmul(
                        psums[s][:],
                        wt[:, 2 * kq:2 * kq + 2, msl],
                        mov[:, 2 * kq:2 * kq + 2, s * NS:(s + 1) * NS],
                        start=start and kq == 0,
                        stop=stop and kq == nq - 1,
                        perf_mode=DR,
                    )

        def mm_bf16(psums, wt, mov, m, nk, start, stop, mov_off=0):
            """bf16-accumulate over nk k-blocks of 128. mov_off: first
            k-block of this weight's bf16 span within the (full) mov tile."""
            msl = slice(m * P, (m + 1) * P)
            for k in range(nk):
                for s in range(SL):
                    nc.tensor.matmul(
                        psums[s][:],
                        wt[:, k, msl],
                        mov[:, mov_off + k, s * NS:(s + 1) * NS],
                        start=start and k == 0,
                        stop=stop and k == nk - 1,
                    )

        # ---- r pass ----
        # Processed in m-PAIRS with wr/ur interleaved per kq, matching the
        # arrival order of the two DMA queues: each kq chunk that lands
        # unlocks 2 m-blocks' worth of matmuls (13.8us of PE work per 6MB
        # delivery window instead of 6.9), nearly hiding the input stream.
        # A pair uses all 8 PSUM banks (2 m x 4 slices).
        for mp in range(MB // 2):
            mpair = (2 * mp, 2 * mp + 1)
            pss = [[pspool.tile([P, NS], F32, tag="ps", name="ps")
                    for _ in range(SL)] for _ in mpair]
            for kq in range(NQ):
                j = slice(2 * kq, 2 * kq + 2)
                for mi, m in enumerate(mpair):
                    msl = slice(m * P, (m + 1) * P)
                    for s in range(SL):
                        nc.tensor.matmul(
                            pss[mi][s][:], w8t["wr"][:, j, msl],
                            xt8[:, j, s * NS:(s + 1) * NS],
                            start=kq == 0, stop=False, perf_mode=DR)
                for mi, m in enumerate(mpair):
                    msl = slice(m * P, (m + 1) * P)
                    for s in range(SL):
                        nc.tensor.matmul(
                            pss[mi][s][:], w8t["ur"][:, j, msl],
                            ht8[:, j, s * NS:(s + 1) * NS],
                            start=False, stop=kq == NQ - 1, perf_mode=DR)
            for mi, m in enumerate(mpair):
                for s in range(SL):
                    rt = rpool.tile([P, NS], BF, tag="r")
                    nc.scalar.activation(rt[:], pss[mi][s][:], AF.Sigmoid,
                                         bias=bt[:, GR * MB + m: GR * MB + m + 1],
                                         scale=ISC)
                    nc.vector.tensor_mul(
                        rht[:, m, s * NS:(s + 1) * NS], rt[:],
                        ht8[:, m, s * NS:(s + 1) * NS])

        # ---- fused h~ / z pass + combine ----
        for m in range(MB):
            msl = slice(m * P, (m + 1) * P)
            hft = hfpool.tile([P, R], F32, tag="hf")
            nc.scalar.dma_start(hft[:], hfd[msl, :])

            psA = [pspool.tile([P, NS], F32, tag="ps", name="psA") for _ in range(SL)]
            mm_fp8(psA, w8t["wh"], xt8, m, NQ8["wh"], start=True, stop=False)
            mm_bf16(psA, wbt["wh"], xtb, m, 2 * (NQ - NQ8["wh"]),
                    start=False, stop=False, mov_off=2 * NQ8["wh"] - 2)
            mm_fp8(psA, w8t["uh"], rht, m, NQ, start=False, stop=True)
            dts = []
            for s in range(SL):
                gt = gpool.tile([P, NS], BF, tag="g")
                nc.scalar.activation(gt[:], psA[s][:], AF.Tanh,
                                     bias=bt[:, GH * MB + m: GH * MB + m + 1],
                                     scale=ISC)
                # g - h does not depend on z: hoist it ahead of the z matmuls
                dt = dtpool.tile([P, NS], F32, tag="dt")
                nc.vector.tensor_sub(dt[:], gt[:], hft[:, s * NS:(s + 1) * NS])
                dts.append(dt)

            psB = [pspool.tile([P, NS], F32, tag="ps", name="psB") for _ in range(SL)]
            if m < MB - 1:
                mm_fp8(psB, w8t["wz"], xt8, m, NQ8["wz"], start=True, stop=False)
                mm_bf16(psB, wbt["wz"], xtb, m, 2 * (NQ - NQ8["wz"]),
                        start=False, stop=False, mov_off=2 * NQ8["wz"] - 2)
                mm_fp8(psB, w8t["uz"], ht8, m, NQ8["uz"], start=False, stop=False)
                mm_bf16(psB, wbt["uz"], htb, m, 2 * (NQ - NQ8["uz"]),
                        start=False, stop=True, mov_off=2 * NQ8["uz"] - 2)
            else:
                # last m: complete each s-slice fully so the z->combine->dma
                # chains drain during (not after) the matmul stream; the
                # per-MM weight reloads are hidden by the background buffer
                msl_ = slice(m * P, (m + 1) * P)
                for s in range(SL):
                    csl = slice(s * NS, (s + 1) * NS)
                    nc.tensor.matmul(psB[s][:], w8t["wz"][:, 0:2, msl_],
                                     xt8[:, 0:2, csl], start=True, stop=False,
                                     perf_mode=DR)
                    for k in range(6):
                        nc.tensor.matmul(psB[s][:], wbt["wz"][:, k, msl_],
                                         xtb[:, k, csl], start=False, stop=False)
                    nc.tensor.matmul(psB[s][:], w8t["uz"][:, 0:2, msl_],
                                     ht8[:, 0:2, csl], start=False, stop=False,
                                     perf_mode=DR)
                    for k in range(6):
                        nc.tensor.matmul(psB[s][:], wbt["uz"][:, k, msl_],
                                         htb[:, k, csl], start=False,
                                         stop=k == 5)
            for s in range(SL):
                ssl = slice(s * NS, (s + 1) * NS)
                zt = rpool.tile([P, NS], BF, tag="z")
                nc.scalar.activation(zt[:], psB[s][:], AF.Sigmoid,
                                     bias=bt[:, GZ * MB + m: GZ * MB + m + 1],
                                     scale=ISC)
                ot = opool.tile([P, NS], BF, tag="o")
                # z*(g-h) ; h + z*(g-h)
                nc.vector.tensor_mul(dts[s][:], zt[:], dts[s][:])
                nc.vector.tensor_add(ot[:], dts[s][:], hft[:, ssl])
                nc.sync.dma_start(outT[msl, ssl], ot[:])

    nc.compile()
    return nc


_NC_CACHE = {}


def _get_nc(R):
    if R not in _NC_CACHE:
        _NC_CACHE[R] = build_nc(R)
    return _NC_CACHE[R]


def blockify(a):
    """[nb*128, C] -> partition-major block layout [128, nb*C]."""
    nb = a.shape[0] // P
    return np.ascontiguousarray(
        a.reshape(nb, P, -1).transpose(1, 0, 2).reshape(P, -1))


def make_in_maps(update, hidden, wz, uz, bz, wr, ur, br, wh, uh, bh,
                 ncores=NCORES):
    wmap = {}
    for nm, w in (("wz", wz), ("uz", uz), ("wr", wr), ("ur", ur),
                  ("wh", wh), ("uh", uh)):
        wT = np.ascontiguousarray(np.asarray(w, np.float32).T) * WSCALE
        nq = NQ8[nm]
        wmap[nm + "8"] = blockify(wT[:nq * 2 * P].astype(f8))
        if nq < NQ:
            wmap[nm + "b"] = blockify(wT[nq * 2 * P:].astype(bf16))
    bias = np.empty((P, 3 * MB), np.float32)
    for g, b in enumerate((bz, br, bh)):
        bias[:, g * MB:(g + 1) * MB] = np.asarray(b, np.float32).reshape(MB, P).T
    rows = update.shape[0]
    rc = rows // ncores
    in_maps = []
    for i in range(ncores):
        sl = slice(i * rc, (i + 1) * rc)
        xT = np.ascontiguousarray(np.asarray(update[sl], np.float32).T)
        hT = np.ascontiguousarray(np.asarray(hidden[sl], np.float32).T)
        in_maps.append(dict(
            x8=blockify(xT.astype(f8)), xb=blockify(xT[2 * P:].astype(bf16)),
            h8=blockify(hT.astype(f8)), hb=blockify(hT[2 * P:].astype(bf16)),
            hf=hT, bias=bias, **wmap))
    return in_maps


def kernel(update, hidden, wz, uz, bz, wr, ur, br, wh, uh, bh):
    global LAST_RESULT
    update = np.asarray(update)
    hidden = np.asarray(hidden)
    R = update.shape[0] // NCORES
    nc = _get_nc(R)
    in_maps = make_in_maps(update, hidden, wz, uz, bz, wr, ur, br, wh, uh, bh)
    res = run_bass_kernel_spmd(nc, in_maps, list(range(NCORES)), trace=TRACE)
    LAST_RESULT = res
    out = np.empty((update.shape[0], H), np.float32)
    for i in range(NCORES):
        out[i * R:(i + 1) * R] = res.results[i]["outT"].T
    return out


# revision 20
# speedup vs baseline: 1.1868x; 1.1868x over previous
"""GRU cell kernel for Trainium2, data-parallel over 8 NeuronCores.

Math (per reference):
    z = sigmoid(x @ wz.T + h @ uz.T + bz)
    r = sigmoid(x @ wr.T + h @ ur.T + br)
    g = tanh(x @ wh.T + (r*h) @ uh.T + bh)
    out = (1-z)*h + z*g = h + z*(g - h)

Everything on-device is computed in TRANSPOSED layout ([feature, row]) so
both matmul operands arrive with the contraction dim on partitions.

Mixed precision: part of the contraction runs as fp8(e4m3) DoubleRow
matmuls (2 MACs/cell/cycle, K=256 per pass), the rest as bf16. Which
k-quarters of each weight matrix are fp8 was chosen by host simulation
to keep max rel err ~0.014 (budget 2e-2):
    wr, ur, uh: all 4 quarters fp8 (r-gate error is attenuated by
        sigmoid slope and the uh moving operand r*h is small in
        magnitude)
    wz, uz, wh: quarter 0 fp8, quarters 1..3 bf16 (z-gate errors are
        amplified by (g - h), tanh has slope 1)
All weights (fp8 and bf16) are pre-scaled by 32 on host (exact in both
formats) so fp8 and bf16 products can share one PSUM accumulation; the
activation undoes it with scale=1/32.

Sharding: rows 16384 -> 8 cores x 2048 rows, weights replicated.
"""

import numpy as np
import ml_dtypes
from contextlib import ExitStack

import concourse.bass as bass
import concourse.bacc as bacc
import concourse.mybir as mybir
import concourse.tile as tile
from concourse.bass_utils import run_bass_kernel_spmd

H = 1024
N_ROWS = 16384
NCORES = 8
P = 128
KB = H // P            # 8 contraction blocks of 128
MB = H // P            # 8 output-feature blocks
NQ = 4                 # k-quarters (256 each)
NS = 512               # rows per matmul moving slice (one PSUM bank)
WSCALE = 32.0          # weight pre-scale (exact power of 2)

# fp8 k-quarters per weight matrix (first nq of 4 quarters are fp8)
NQ8 = {"wz": 1, "uz": 1, "wr": 4, "ur": 4, "wh": 2, "uh": 4}

BF = mybir.dt.bfloat16
F8 = mybir.dt.float8e4
F32 = mybir.dt.float32
AF = mybir.ActivationFunctionType
DR = mybir.MatmulPerfMode.DoubleRow
bf16 = ml_dtypes.bfloat16
f8 = ml_dtypes.float8_e4m3

# Set by test harness to capture a trace; harness-facing default off.
TRACE = False
LAST_RESULT = None


def build_nc(R=N_ROWS // NCORES):
    """Build the per-core Bass program. R rows per core, single chunk."""
    SL = R // NS           # moving slices (4 for R=2048)

    nc = bacc.Bacc(trn_type="TRN2", target_bir_lowering=False,
                   debug=False, enable_asserts=False)

    # All block tensors use "partition-major block layout": [128, nblk*cols]
    # with element (p, k*cols + c) = T[k*128 + p, c]. One DMA descriptor can
    # then cover any k-block range (descriptor processing on the queue
    # engines costs ~0.65us each — fine-grained DMA was the startup limiter).
    x8d = nc.dram_tensor("x8", [P, KB * R], F8, kind="ExternalInput").ap()
    xbd = nc.dram_tensor("xb", [P, 6 * R], BF, kind="ExternalInput").ap()
    h8d = nc.dram_tensor("h8", [P, KB * R], F8, kind="ExternalInput").ap()
    hbd = nc.dram_tensor("hb", [P, 6 * R], BF, kind="ExternalInput").ap()
    hfd = nc.dram_tensor("hf", [H, R], F32, kind="ExternalInput").ap()
    w8d = {}
    wbd = {}
    for nm, nq in NQ8.items():
        w8d[nm] = nc.dram_tensor(nm + "8", [P, nq * 2 * H], F8,
                                 kind="ExternalInput").ap()
        if nq < NQ:
            wbd[nm] = nc.dram_tensor(nm + "b", [P, (NQ - nq) * 2 * H], BF,
                                     kind="ExternalInput").ap()
    bias = nc.dram_tensor("bias", [P, 3 * MB], F32, kind="ExternalInput").ap()
    outT = nc.dram_tensor("outT", [H, R], BF, kind="ExternalOutput").ap()

    with tile.TileContext(nc) as tc, ExitStack() as ctx:
        wpool = ctx.enter_context(tc.tile_pool(name="w", bufs=3))
        dpool = ctx.enter_context(tc.tile_pool(name="d", bufs=1))
        hfpool = ctx.enter_context(tc.tile_pool(name="hf", bufs=2))
        rpool = ctx.enter_context(tc.tile_pool(name="r", bufs=3))
        gpool = ctx.enter_context(tc.tile_pool(name="g", bufs=3))
        dtpool = ctx.enter_context(tc.tile_pool(name="dt", bufs=SL))
        opool = ctx.enter_context(tc.tile_pool(name="o", bufs=4))
        cpool = ctx.enter_context(tc.tile_pool(name="c", bufs=1))
        pspool = ctx.enter_context(tc.tile_pool(name="ps", bufs=8, space="PSUM"))

        # Two HW DMA queues: sync (bulk prefetch + output) and scalar
        # (x8/h8 at startup, then the latency-sensitive hf stream). Keeping
        # the hf stream off the bulk queue is essential — behind the 12MB
        # hz prefetch it starves the r-pass rh-multiply for ~30us.
        bt = cpool.tile([P, 3 * MB], F32, tag="bias")
        nc.scalar.dma_start(bt[:], bias[:])
        # bias column layout: [z:0..7 | r:8..15 | h:16..23]
        GZ, GR, GH = 0, 1, 2
        ISC = 1.0 / WSCALE

        # ---- SBUF data tiles + DMA in consumption order ----
        # r-pass m=0 consumes wr/x first, then ur/h; interleave so the PE
        # can start as soon as the first (weight, data) pair lands.
        xt8 = dpool.tile([P, KB, R], F8, tag="x8")
        ht8 = dpool.tile([P, KB, R], F8, tag="h8")
        xtb = dpool.tile([P, 6, R], BF, tag="xb")
        htb = dpool.tile([P, 6, R], BF, tag="hb")
        rht = dpool.tile([P, KB, R], F8, tag="rh")

        w8t = {}
        wbt = {}
        # Critical path (r-pass m=0): kq-granular (0.25-0.5MB) descriptors,
        # weights on the sync queue, x8/h8 on the scalar queue, in the exact
        # consumption order of the first psum group.
        # Critical 6MB split evenly across the two HW queues (each delivers
        # ~270GB/s): sync gets wr+h8, scalar gets x8+ur, both in the exact
        # consumption order of the first psum group.
        w8t["wr"] = wpool.tile([P, KB, H], F8, tag="w8", name="wr8", bufs=2)
        w8t["ur"] = wpool.tile([P, KB, H], F8, tag="w8", name="ur8", bufs=2)
        for kq in range(NQ):
            j = slice(2 * kq, 2 * kq + 2)
            nc.sync.dma_start(w8t["wr"][:, j, :], w8d["wr"][:, 2 * kq * H:(2 * kq + 2) * H])
            nc.scalar.dma_start(xt8[:, j, :], x8d[:, 2 * kq * R:(2 * kq + 2) * R])
            nc.sync.dma_start(ht8[:, j, :], h8d[:, 2 * kq * R:(2 * kq + 2) * R])
            nc.scalar.dma_start(w8t["ur"][:, j, :], w8d["ur"][:, 2 * kq * H:(2 * kq + 2) * H])

        # Warm up the ACT table set (sigmoid_and_others covers tanh too) on an
        # instruction with minimal sync waits — walrus can't attach the
        # PSEUDO_LOAD_ACT_FUNC_SET to an activation that already carries two
        # sem waits ("Too many sync wait commands"). Emitted after the
        # critical DMAs so the 2x1.3us table loads don't delay them.
        warm = cpool.tile([P, 8], F32, tag="warm")
        nc.gpsimd.memset(warm[:], 0.0)
        nc.scalar.activation(warm[:], warm[:], AF.Sigmoid)

        # hz-pass weights + bf16 moving data: one descriptor per tensor,
        # streamed during the r-pass on the sync queue.
        for nm in ("wh", "wz", "uz"):
            nq = NQ8[nm]
            w8t[nm] = wpool.tile([P, 2 * nq, H], F8, tag="w8q", name=nm + "8")
            nc.sync.dma_start(w8t[nm][:, :, :], w8d[nm][:, :])
            wbt[nm] = wpool.tile([P, 2 * (NQ - nq), H], BF, tag="wbq", name=nm + "b")
            nc.sync.dma_start(wbt[nm][:, :, :], wbd[nm][:, :])
        nc.sync.dma_start(xtb[:, :, :], xbd[:, :])
        nc.sync.dma_start(htb[:, :, :], hbd[:, :])
        # uh8 reuses wr's buffer (tag w8, bufs=2): its DMA waits for the
        # r-pass to drain, so it must sit at the TAIL of the sync queue to
        # avoid head-of-line blocking the prefetch above.
        w8t["uh"] = wpool.tile([P, KB, H], F8, tag="w8", name="uh8", bufs=2)
        nc.sync.dma_start(w8t["uh"][:, :, :], w8d["uh"][:, :])


        def mm_fp8(psums, wt, mov, m, nq, start, stop):
            """DoubleRow-accumulate wt.T @ mov for feature block m over
            fp8 k-quarters 0..nq-1."""
            msl = slice(m * P, (m + 1) * P)
            for kq in range(nq):
                for s in range(SL):
                    nc.tensor.matmul(
                        psums[s][:],
                        wt[:, 2 * kq:2 * kq + 2, msl],
                        mov[:, 2 * kq:2 * kq + 2, s * NS:(s + 1) * NS],
                        start=start and kq == 0,
                        stop=stop and kq == nq - 1,
                        perf_mode=DR,
                    )

        def mm_bf16(psums, wt, mov, m, nk, start, stop, mov_off=0):
            """bf16-accumulate over nk k-blocks of 128. mov_off: first
            k-block of this weight's bf16 span within the (full) mov tile."""
            msl = slice(m * P, (m + 1) * P)
            for k in range(nk):
                for s in range(SL):
                    nc.tensor.matmul(
                        psums[s][:],
                        wt[:, k, msl],
                        mov[:, mov_off + k, s * NS:(s + 1) * NS],
                        start=start and k == 0,
                        stop=stop and k == nk - 1,
                    )

        # ---- r pass ----
        # wr/ur interleaved per kq: matches the arrival order of the two DMA
        # queues so the m-groups consume data as it lands. The FIRST two
        # m-blocks are fused into one kq-interleaved wave: during the 0-17us
        # window the critical 6MB is still streaming in and a single m-group
        # (6.9us of matmuls) cannot cover the delivery time; two can. Later
        # groups stay single-m so their ACT drain pipelines under the next
        # group's matmuls (4+4 PSUM bank split — fusing ALL pairs regresses).
        def r_mms(ms, pss):
            for kq in range(NQ):
                j = slice(2 * kq, 2 * kq + 2)
                for mi, m in enumerate(ms):
                    msl = slice(m * P, (m + 1) * P)
                    for s in range(SL):
                        nc.tensor.matmul(
                            pss[mi][s][:], w8t["wr"][:, j, msl],
                            xt8[:, j, s * NS:(s + 1) * NS],
                            start=kq == 0, stop=False, perf_mode=DR)
                for mi, m in enumerate(ms):
                    msl = slice(m * P, (m + 1) * P)
                    for s in range(SL):
                        nc.tensor.matmul(
                            pss[mi][s][:], w8t["ur"][:, j, msl],
                            ht8[:, j, s * NS:(s + 1) * NS],
                            start=False, stop=kq == NQ - 1, perf_mode=DR)

        def r_acts(ms, pss):
            for mi, m in enumerate(ms):
                for s in range(SL):
                    rt = rpool.tile([P, NS], BF, tag="r")
                    nc.scalar.activation(rt[:], pss[mi][s][:], AF.Sigmoid,
                                         bias=bt[:, GR * MB + m: GR * MB + m + 1],
                                         scale=ISC)
                    nc.vector.tensor_mul(
                        rht[:, m, s * NS:(s + 1) * NS], rt[:],
                        ht8[:, m, s * NS:(s + 1) * NS])

        pss01 = [[pspool.tile([P, NS], F32, tag="ps", name="ps")
                  for _ in range(SL)] for _ in range(2)]
        r_mms([0, 1], pss01)
        r_acts([0, 1], pss01)
        for m in range(2, MB):
            ps = [pspool.tile([P, NS], F32, tag="ps", name="ps") for _ in range(SL)]
            r_mms([m], [ps])
            r_acts([m], [ps])

        # ---- fused h~ / z pass + combine ----
        for m in range(MB):
            msl = slice(m * P, (m + 1) * P)
            hft = hfpool.tile([P, R], F32, tag="hf")
            nc.scalar.dma_start(hft[:], hfd[msl, :])

            psA = [pspool.tile([P, NS], F32, tag="ps", name="psA") for _ in range(SL)]
            mm_fp8(psA, w8t["wh"], xt8, m, NQ8["wh"], start=True, stop=False)
            mm_bf16(psA, wbt["wh"], xtb, m, 2 * (NQ - NQ8["wh"]),
                    start=False, stop=False, mov_off=2 * NQ8["wh"] - 2)
            mm_fp8(psA, w8t["uh"], rht, m, NQ, start=False, stop=True)
            dts = []
            for s in range(SL):
                gt = gpool.tile([P, NS], BF, tag="g")
                nc.scalar.activation(gt[:], psA[s][:], AF.Tanh,
                                     bias=bt[:, GH * MB + m: GH * MB + m + 1],
                                     scale=ISC)
                # g - h does not depend on z: hoist it ahead of the z matmuls
                dt = dtpool.tile([P, NS], F32, tag="dt")
                nc.vector.tensor_sub(dt[:], gt[:], hft[:, s * NS:(s + 1) * NS])
                dts.append(dt)

            psB = [pspool.tile([P, NS], F32, tag="ps", name="psB") for _ in range(SL)]
            if m < MB - 1:
                mm_fp8(psB, w8t["wz"], xt8, m, NQ8["wz"], start=True, stop=False)
                mm_bf16(psB, wbt["wz"], xtb, m, 2 * (NQ - NQ8["wz"]),
                        start=False, stop=False, mov_off=2 * NQ8["wz"] - 2)
                mm_fp8(psB, w8t["uz"], ht8, m, NQ8["uz"], start=False, stop=False)
                mm_bf16(psB, wbt["uz"], htb, m, 2 * (NQ - NQ8["uz"]),
                        start=False, stop=True, mov_off=2 * NQ8["uz"] - 2)
            else:
                # last m: complete each s-slice fully so the z->combine->dma
                # chains drain during (not after) the matmul stream; the
                # per-MM weight reloads are hidden by the background buffer
                msl_ = slice(m * P, (m + 1) * P)
                for s in range(SL):
                    csl = slice(s * NS, (s + 1) * NS)
                    nc.tensor.matmul(psB[s][:], w8t["wz"][:, 0:2, msl_],
                                     xt8[:, 0:2, csl], start=True, stop=False,
                                     perf_mode=DR)
                    for k in range(6):
                        nc.tensor.matmul(psB[s][:], wbt["wz"][:, k, msl_],
                                         xtb[:, k, csl], start=False, stop=False)
                    nc.tensor.matmul(psB[s][:], w8t["uz"][:, 0:2, msl_],
                                     ht8[:, 0:2, csl], start=False, stop=False,
                                     perf_mode=DR)
                    for k in range(6):
                        nc.tensor.matmul(psB[s][:], wbt["uz"][:, k, msl_],
                                         htb[:, k, csl], start=False,
                                         stop=k == 5)
            for s in range(SL):
                ssl = slice(s * NS, (s + 1) * NS)
                zt = rpool.tile([P, NS], BF, tag="z")
                nc.scalar.activation(zt[:], psB[s][:], AF.Sigmoid,
                                     bias=bt[:, GZ * MB + m: GZ * MB + m + 1],
                                     scale=ISC)
                ot = opool.tile([P, NS], BF, tag="o")
                # z*(g-h) ; h + z*(g-h)
                nc.vector.tensor_mul(dts[s][:], zt[:], dts[s][:])
                nc.vector.tensor_add(ot[:], dts[s][:], hft[:, ssl])
                nc.sync.dma_start(outT[msl, ssl], ot[:])

    nc.compile()
    return nc


_NC_CACHE = {}


def _get_nc(R):
    if R not in _NC_CACHE:
        _NC_CACHE[R] = build_nc(R)
    return _NC_CACHE[R]


def blockify(a):
    """[nb*128, C] -> partition-major block layout [128, nb*C]."""
    nb = a.shape[0] // P
    return np.ascontiguousarray(
        a.reshape(nb, P, -1).transpose(1, 0, 2).reshape(P, -1))


def make_in_maps(update, hidden, wz, uz, bz, wr, ur, br, wh, uh, bh,
                 ncores=NCORES):
    wmap = {}
    for nm, w in (("wz", wz), ("uz", uz), ("wr", wr), ("ur", ur),
                  ("wh", wh), ("uh", uh)):
        wT = np.ascontiguousarray(np.asarray(w, np.float32).T) * WSCALE
        nq = NQ8[nm]
        wmap[nm + "8"] = blockify(wT[:nq * 2 * P].astype(f8))
        if nq < NQ:
            wmap[nm + "b"] = blockify(wT[nq * 2 * P:].astype(bf16))
    bias = np.empty((P, 3 * MB), np.float32)
    for g, b in enumerate((bz, br, bh)):
        bias[:, g * MB:(g + 1) * MB] = np.asarray(b, np.float32).reshape(MB, P).T
    rows = update.shape[0]
    rc = rows // ncores
    in_maps = []
    for i in range(ncores):
        sl = slice(i * rc, (i + 1) * rc)
        xT = np.ascontiguousarray(np.asarray(update[sl], np.float32).T)
        hT = np.ascontiguousarray(np.asarray(hidden[sl], np.float32).T)
        in_maps.append(dict(
            x8=blockify(xT.astype(f8)), xb=blockify(xT[2 * P:].astype(bf16)),
            h8=blockify(hT.astype(f8)), hb=blockify(hT[2 * P:].astype(bf16)),
            hf=hT, bias=bias, **wmap))
    return in_maps


def kernel(update, hidden, wz, uz, bz, wr, ur, br, wh, uh, bh):
    global LAST_RESULT
    update = np.asarray(update)
    hidden = np.asarray(hidden)
    R = update.shape[0] // NCORES
    nc = _get_nc(R)
    in_maps = make_in_maps(update, hidden, wz, uz, bz, wr, ur, br, wh, uh, bh)
    res = run_bass_kernel_spmd(nc, in_maps, list(range(NCORES)), trace=TRACE)
    LAST_RESULT = res
    out = np.empty((update.shape[0], H), np.float32)
    for i in range(NCORES):
        out[i * R:(i + 1) * R] = res.results[i]["outT"].T
    return out
